# revision 1
# baseline (speedup 1.0000x reference)
"""NetVLAD Trainium2 Bass kernel, SPMD over 8 NeuronCores.

Contract: kernel(x, Wc, C) takes the FULL inputs
  x  [16, 56, 56, 512] f32, Wc [512, 32] f32, C [512, 32] f32
and returns the FULL output [16, 16384] f32 (matches reference()).

Sharding: data-parallel over batch — 2 samples per core; Wc/C replicated.

Per-core algorithm (3136 pixels/sample, D=512, K=32; 49 tiles of 128
pixels x 512 dims):
  - load x tile [128, 512] (f32r) — contiguous 256KB DMAs
  - PE transpose (4x 128x128 via identity) -> xT in PSUM -> copy to SBUF
  - mm1: s[n,k] = xT.T @ Wc (4 accumulating matmuls, f32r)
  - softmax over k without max-subtraction (|s| <= ~10, exp-safe):
    ACT Exp with fused row-sum accumulator, DVE reciprocal + scale
  - mm2: axT[k,d] += a.T @ x (a stationary 32 cols, x streams N=512, f32r)
    and a_sum[k] += a.T @ ones into a second PSUM bank
  - per-sample epilogue: vT = axT + C^T * a_sum, PE-transpose back to
    [d,k], fused intra+global L2 normalization (the global L2 norm of the
    intra-normalized matrix is exactly sqrt(512), folded analytically),
    DMA out
float32r is used on the matmul paths (1 cycle/row at N>=256 vs 4 for
fp32); measured end-to-end relative error vs the fp32 reference ~1e-4.
"""
import sys

if '/opt/trn_rl_repo' not in sys.path:
    sys.path.insert(0, '/opt/trn_rl_repo')

from contextlib import ExitStack

import numpy as np

F32 = None  # filled on first build (lazy imports keep module import cheap)

N_PIX = 3136
N_SAMP = 2
N_ROWS = N_PIX * N_SAMP
P = 128
NT = N_ROWS // P      # 49
D = 512
K = 32
DC = D // P           # 4
BOUND_T = N_PIX // P  # 24
BOUND_R = N_PIX - BOUND_T * P  # 64
N_CORES = 8

_cache = {}


def _build(use_f32r=True, copy_split=3):
    import concourse.bacc as bacc
    import concourse.mybir as mybir
    import concourse.tile as tile
    from concourse.bass import ts

    F32 = mybir.dt.float32
    F32R = mybir.dt.float32r
    DT = F32R if use_f32r else F32

    nc = bacc.Bacc("TRN2", target_bir_lowering=False, debug=False)

    x = nc.declare_dram_parameter("x", [N_ROWS, D], DT, isOutput=False)
    wc = nc.declare_dram_parameter("wc", [D, K], DT, isOutput=False)
    ct = nc.declare_dram_parameter("ct", [K, D], F32, isOutput=False)
    ident = nc.declare_dram_parameter("ident", [P, P], DT, isOutput=False)
    id32 = nc.declare_dram_parameter("id32", [K, K], F32, isOutput=False)
    ones2 = nc.declare_dram_parameter("ones2", [P, 2], DT, isOutput=False)
    out = nc.declare_dram_parameter("out", [N_SAMP, DC, P, K], F32,
                                    isOutput=True)
    x, wc, ct, ident, out, id32, ones2 = (x.ap(), wc.ap(), ct.ap(),
                                          ident.ap(), out.ap(), id32.ap(),
                                          ones2.ap())

    with tile.TileContext(nc) as tc, ExitStack() as ctx:
        consts = ctx.enter_context(tc.tile_pool(name="consts", bufs=1))
        xpool = ctx.enter_context(tc.tile_pool(name="xpool", bufs=4))
        xtpool = ctx.enter_context(tc.tile_pool(name="xtpool", bufs=3))
        small = ctx.enter_context(tc.tile_pool(name="small", bufs=4))
        epil = ctx.enter_context(tc.tile_pool(name="epil", bufs=2))
        ps_big = ctx.enter_context(tc.tile_pool(name="ps_big", bufs=2,
                                                space="PSUM"))
        ps_sm = ctx.enter_context(tc.tile_pool(name="ps_sm", bufs=2,
                                               space="PSUM"))
        ps_acc = ctx.enter_context(tc.tile_pool(name="ps_acc", bufs=2,
                                                space="PSUM"))

        wc_sb = consts.tile([P, DC, K], DT)
        nc.sync.dma_start(out=wc_sb, in_=wc.rearrange("(c p) k -> p c k", p=P))
        ct_sb = consts.tile([K, D], F32)
        nc.sync.dma_start(out=ct_sb, in_=ct)
        id_sb = consts.tile([P, P], DT)
        nc.sync.dma_start(out=id_sb, in_=ident)
        id32_sb = consts.tile([K, K], F32)
        nc.sync.dma_start(out=id32_sb, in_=id32)
        ones_sb = consts.tile([P, 2], DT)
        nc.sync.dma_start(out=ones_sb, in_=ones2)

        acc = [ps_acc.tile([K, D], F32, name=f"acc{s}", tag="acc")
               for s in range(N_SAMP)]
        asum_ps = [ps_acc.tile([K, 2], F32, name=f"asumps{s}", tag="asum_ps")
                   for s in range(N_SAMP)]
        started = [False, False]

        def epilogue(s):
            asum_sb = epil.tile([K, 1], F32, name=f"asum{s}", tag="asum")
            nc.vector.tensor_copy(asum_sb, asum_ps[s][:, 0:1])
            vt_sb = epil.tile([K, D], F32, name=f"vt{s}", tag="vt")
            nc.vector.tensor_scalar_mul(vt_sb, ct_sb, asum_sb)
            nc.vector.tensor_add(vt_sb, vt_sb, acc[s][:, :])
            v_ps = ps_sm.tile([P, DC, K], F32, name=f"vps{s}", tag="sps")
            for j in range(DC):
                nc.tensor.transpose(v_ps[:, j, :], vt_sb[:, ts(j, P)], id32_sb)
            v_sb = epil.tile([P, DC, K], F32, name=f"v{s}", tag="v")
            nc.vector.tensor_copy(v_sb, v_ps)
            vsq = epil.tile([P, DC, K], F32, name=f"vsq{s}", tag="vsq")
            nc.vector.tensor_mul(vsq, v_sb, v_sb)
            ssq = epil.tile([P, DC], F32, name=f"ssq{s}", tag="ssq")
            nc.vector.reduce_sum(ssq, vsq, axis=mybir.AxisListType.X)
            snorm = epil.tile([P, DC], F32, name=f"sn{s}", tag="sn")
            nc.scalar.activation(snorm, ssq,
                                 mybir.ActivationFunctionType.Sqrt,
                                 scale=float(D))
            rmult = epil.tile([P, DC], F32, name=f"rm{s}", tag="rm")
            nc.vector.reciprocal(rmult, snorm)
            for j in range(DC):
                nc.vector.tensor_scalar_mul(v_sb[:, j, :], v_sb[:, j, :],
                                            rmult[:, j:j + 1])
            nc.sync.dma_start(out=out[s].rearrange("c p k -> p c k"),
                              in_=v_sb)

        for t in range(NT):
            x_t = xpool.tile([P, D], DT, name="x_t")
            nc.sync.dma_start(out=x_t, in_=x[ts(t, P), :])

            xT_ps = ps_big.tile([P, DC, P], DT, name="xT_ps")
            for j in range(DC):
                nc.tensor.transpose(xT_ps[:, j, :], x_t[:, ts(j, P)], id_sb)
            xT_sb = xtpool.tile([P, DC, P], DT, name="xT_sb")
            if copy_split > 0:
                nc.vector.tensor_copy(xT_sb[:, 0:copy_split, :],
                                      xT_ps[:, 0:copy_split, :])
            if copy_split < DC:
                nc.scalar.copy(xT_sb[:, copy_split:DC, :],
                               xT_ps[:, copy_split:DC, :])

            s_ps = ps_sm.tile([P, K], F32, name="s_ps", tag="sps")
            for j in range(DC):
                nc.tensor.matmul(s_ps, xT_sb[:, j, :], wc_sb[:, j, :],
                                 start=(j == 0), stop=(j == DC - 1))

            exp_sb = small.tile([P, K], F32, name="exp_sb")
            sumx = small.tile([P, 1], F32, name="sumx")
            nc.scalar.activation(exp_sb, s_ps,
                                 mybir.ActivationFunctionType.Exp,
                                 accum_out=sumx)
            rcp = small.tile([P, 1], F32, name="rcp")
            nc.vector.reciprocal(rcp, sumx)
            a_sb = small.tile([P, K], DT, name="a_sb")
            nc.vector.tensor_scalar_mul(a_sb, exp_sb, rcp)

            if t < BOUND_T:
                parts = [(0, 0, P)]
            elif t == BOUND_T:
                parts = [(0, 0, BOUND_R), (1, BOUND_R, P)]
            else:
                parts = [(1, 0, P)]
            for s, r0, r1 in parts:
                first = not started[s]
                started[s] = True
                last_tile = (t == BOUND_T and s == 0) or \
                            (t == NT - 1 and s == 1)
                nc.tensor.matmul(acc[s][:, :], a_sb[r0:r1, :], x_t[r0:r1, :],
                                 start=first, stop=last_tile,
                                 skip_group_check=True)
                nc.tensor.matmul(asum_ps[s][:, :], a_sb[r0:r1, :],
                                 ones_sb[r0:r1, :],
                                 start=first, stop=last_tile,
                                 skip_group_check=True)
                if last_tile:
                    epilogue(s)

    nc.finalize()
    return nc


def _get_nc():
    if "nc" not in _cache:
        _cache["nc"] = _build()
    return _cache["nc"]


def kernel(x, Wc, C):
    from concourse.bass_utils import run_bass_kernel_spmd

    nc = _get_nc()

    x = np.asarray(x, dtype=np.float32)
    Wc = np.ascontiguousarray(np.asarray(Wc, dtype=np.float32))
    ct = np.ascontiguousarray(np.asarray(C, dtype=np.float32).T)
    ident = np.eye(P, dtype=np.float32)
    id32 = np.eye(K, dtype=np.float32)
    ones2 = np.ones((P, 2), dtype=np.float32)

    B = x.shape[0]
    per = B // N_CORES
    maps = []
    for i in range(N_CORES):
        xs = np.ascontiguousarray(
            x[i * per:(i + 1) * per].reshape(N_ROWS, D))
        maps.append({"x": xs, "wc": Wc, "ct": ct, "ident": ident,
                     "id32": id32, "ones2": ones2})

    res = run_bass_kernel_spmd(nc, maps, list(range(N_CORES)))
    outs = [r["out"].reshape(N_SAMP, D * K) for r in res.results]
    return np.concatenate(outs, axis=0)



# revision 3
# speedup vs baseline: 1.4428x; 1.4428x over previous
"""NetVLAD Trainium2 Bass kernel, SPMD over 8 NeuronCores.

Contract: kernel(x, Wc, C) takes the FULL inputs
  x  [16, 56, 56, 512] f32, Wc [512, 32] f32, C [512, 32] f32
and returns the FULL output [16, 16384] f32 (matches reference()).

Sharding: data-parallel over batch — 2 samples per core; Wc/C replicated.

Per-core algorithm (3136 pixels/sample, D=512, K=32; 49 tiles of 128
pixels x 512 dims), all matmul paths in bf16 (PSUM accumulation f32;
measured end-to-end rel err ~2e-3 vs the f32 reference):
  - load x tile [128, 512] f32 (contiguous 256KB DMA)
  - convert f32 -> bf16 split across Scalar (1 chunk) + GpSimd (3 chunks)
  - PE transpose (4x 128x128 bf16 via identity) -> xT in PSUM -> DVE copy
  - mm1: s[n,k] = xT.T @ Wc (4 accumulating bf16 matmuls)
  - softmax over k (no max-subtraction; |s| small): ACT Exp, DVE row-sum,
    DVE reciprocal, DVE scale -> a in bf16
  - mm2: ax[k,d] += a.T @ x16 and a_sum[k] += a.T @ ones (PSUM, f32)
  - epilogue: vT = axT + C^T * a_sum, PE-transpose back to [d,k], fused
    intra+global L2 normalization (global norm of the intra-normalized
    matrix is exactly sqrt(512), folded analytically); Sqrt deferred to
    the end for both samples so the ACT Exp/Sqrt table loads once.
The emission is software-pipelined with a 4-7 iteration skew
(load t / convert t-4 / transpose t-5 / mm1+softmax t-6 / mm2 t-7) so
the in-order PE never head-of-line blocks on the softmax chain.
"""
import sys

if '/opt/trn_rl_repo' not in sys.path:
    sys.path.insert(0, '/opt/trn_rl_repo')

from contextlib import ExitStack

import numpy as np

N_PIX = 3136
N_SAMP = 2
N_ROWS = N_PIX * N_SAMP
P = 128
NT = N_ROWS // P      # 49
D = 512
K = 32
DC = D // P           # 4
BOUND_T = N_PIX // P  # 24
BOUND_R = N_PIX - BOUND_T * P  # 64
N_CORES = 8

_cache = {}


def _build():
    import concourse.bacc as bacc
    import concourse.mybir as mybir
    import concourse.tile as tile
    from concourse.bass import ts

    F32 = mybir.dt.float32
    BF16 = mybir.dt.bfloat16

    nc = bacc.Bacc("TRN2", target_bir_lowering=False, debug=False)

    x = nc.declare_dram_parameter("x", [N_ROWS, D], F32, isOutput=False)
    wc = nc.declare_dram_parameter("wc16", [D, K], BF16, isOutput=False)
    ct = nc.declare_dram_parameter("ct", [K, D], F32, isOutput=False)
    ident = nc.declare_dram_parameter("ident16", [P, P], BF16, isOutput=False)
    id32 = nc.declare_dram_parameter("id32", [K, K], F32, isOutput=False)
    ones2 = nc.declare_dram_parameter("ones16", [P, 2], BF16, isOutput=False)
    out = nc.declare_dram_parameter("out", [N_SAMP, DC, P, K], F32,
                                    isOutput=True)
    x, wc, ct, ident, out, id32, ones2 = (x.ap(), wc.ap(), ct.ap(),
                                          ident.ap(), out.ap(), id32.ap(),
                                          ones2.ap())

    with tile.TileContext(nc) as tc, ExitStack() as ctx:
        consts = ctx.enter_context(tc.tile_pool(name="consts", bufs=1))
        xpool = ctx.enter_context(tc.tile_pool(name="xpool", bufs=6))
        x16pool = ctx.enter_context(tc.tile_pool(name="x16pool", bufs=6))
        xtpool = ctx.enter_context(tc.tile_pool(name="xtpool", bufs=3))
        small = ctx.enter_context(tc.tile_pool(name="small", bufs=3))
        apool = ctx.enter_context(tc.tile_pool(name="apool", bufs=3))
        epil = ctx.enter_context(tc.tile_pool(name="epil", bufs=2))
        ps_big = ctx.enter_context(tc.tile_pool(name="ps_big", bufs=2,
                                                space="PSUM"))
        ps_sm = ctx.enter_context(tc.tile_pool(name="ps_sm", bufs=2,
                                               space="PSUM"))
        ps_acc = ctx.enter_context(tc.tile_pool(name="ps_acc", bufs=2,
                                                space="PSUM"))

        wc_sb = consts.tile([P, DC, K], BF16)
        nc.sync.dma_start(out=wc_sb, in_=wc.rearrange("(c p) k -> p c k", p=P))
        ct_sb = consts.tile([K, D], F32)
        nc.sync.dma_start(out=ct_sb, in_=ct)
        id_sb = consts.tile([P, P], BF16)
        nc.sync.dma_start(out=id_sb, in_=ident)
        id32_sb = consts.tile([K, K], F32)
        nc.sync.dma_start(out=id32_sb, in_=id32)
        ones_sb = consts.tile([P, 2], BF16)
        nc.sync.dma_start(out=ones_sb, in_=ones2)

        acc = [ps_acc.tile([K, D], F32, name=f"acc{s}", tag="acc")
               for s in range(N_SAMP)]
        asum_ps = [ps_acc.tile([K, 2], F32, name=f"asumps{s}", tag="asum_ps")
                   for s in range(N_SAMP)]
        started = [False, False]

        xs, x16s, xTs, a16s = {}, {}, {}, {}
        ep_state = {}

        def load(t):
            x_t = xpool.tile([P, D], F32, name="x_t")
            nc.sync.dma_start(out=x_t, in_=x[ts(t, P), :])
            xs[t] = x_t

        def convert(t):
            x16 = x16pool.tile([P, D], BF16, name="x16")
            nc.scalar.copy(x16[:, 0:P], xs[t][:, 0:P])
            nc.gpsimd.tensor_copy(x16[:, P:D], xs[t][:, P:D])
            x16s[t] = x16

        def transpose_stage(t):
            xT_ps = ps_big.tile([P, DC, P], BF16, name="xT_ps")
            for j in range(DC):
                nc.tensor.transpose(xT_ps[:, j, :], x16s[t][:, ts(j, P)],
                                    id_sb)
            xT_sb = xtpool.tile([P, DC, P], BF16, name="xT_sb")
            nc.vector.tensor_copy(xT_sb, xT_ps)
            xTs[t] = xT_sb

        def mm1_softmax(t):
            s_ps = ps_sm.tile([P, K], F32, name="s_ps", tag="sps")
            for j in range(DC):
                nc.tensor.matmul(s_ps, xTs[t][:, j, :], wc_sb[:, j, :],
                                 start=(j == 0), stop=(j == DC - 1))
            exp_sb = small.tile([P, K], F32, name="exp_sb")
            nc.scalar.activation(exp_sb, s_ps,
                                 mybir.ActivationFunctionType.Exp)
            sumx = small.tile([P, 1], F32, name="sumx")
            nc.vector.reduce_sum(sumx, exp_sb, axis=mybir.AxisListType.X)
            rcp = small.tile([P, 1], F32, name="rcp")
            nc.vector.reciprocal(rcp, sumx)
            a16 = apool.tile([P, K], BF16, name="a16")
            nc.vector.tensor_scalar_mul(a16, exp_sb, rcp)
            a16s[t] = a16

        def epilogue_a(s):
            asum_sb = epil.tile([K, 1], F32, name=f"asum{s}", tag="asum")
            nc.vector.tensor_copy(asum_sb, asum_ps[s][:, 0:1])
            vt_sb = epil.tile([K, D], F32, name=f"vt{s}", tag="vt")
            nc.vector.tensor_scalar_mul(vt_sb, ct_sb, asum_sb)
            nc.vector.tensor_add(vt_sb, vt_sb, acc[s][:, :])
            v_ps = ps_sm.tile([P, DC, K], F32, name=f"vps{s}", tag="sps")
            for j in range(DC):
                nc.tensor.transpose(v_ps[:, j, :], vt_sb[:, ts(j, P)],
                                    id32_sb)
            v_sb = epil.tile([P, DC, K], F32, name=f"v{s}", tag="v")
            nc.vector.tensor_copy(v_sb, v_ps)
            vsq = epil.tile([P, DC, K], F32, name=f"vsq{s}", tag="vsq")
            nc.vector.tensor_mul(vsq, v_sb, v_sb)
            ssq = epil.tile([P, DC], F32, name=f"ssq{s}", tag="ssq")
            nc.vector.reduce_sum(ssq, vsq, axis=mybir.AxisListType.X)
            ep_state[s] = (v_sb, ssq)

        def epilogue_b(s):
            v_sb, ssq = ep_state[s]
            snorm = epil.tile([P, DC], F32, name=f"sn{s}", tag="sn")
            nc.scalar.activation(snorm, ssq,
                                 mybir.ActivationFunctionType.Sqrt,
                                 scale=float(D))
            rmult = epil.tile([P, DC], F32, name=f"rm{s}", tag="rm")
            nc.vector.reciprocal(rmult, snorm)
            for j in range(DC):
                nc.vector.tensor_scalar_mul(v_sb[:, j, :], v_sb[:, j, :],
                                            rmult[:, j:j + 1])
            nc.sync.dma_start(out=out[s].rearrange("c p k -> p c k"),
                              in_=v_sb)

        def mm2_stage(t):
            if t < BOUND_T:
                parts = [(0, 0, P)]
            elif t == BOUND_T:
                parts = [(0, 0, BOUND_R), (1, BOUND_R, P)]
            else:
                parts = [(1, 0, P)]
            a16 = a16s[t]
            for s, r0, r1 in parts:
                first = not started[s]
                started[s] = True
                last_tile = (t == BOUND_T and s == 0) or \
                            (t == NT - 1 and s == 1)
                nc.tensor.matmul(acc[s][:, :], a16[r0:r1, :],
                                 x16s[t][r0:r1, :],
                                 start=first, stop=last_tile,
                                 skip_group_check=True)
                nc.tensor.matmul(asum_ps[s][:, :], a16[r0:r1, :],
                                 ones_sb[r0:r1, :],
                                 start=first, stop=last_tile,
                                 skip_group_check=True)
                if last_tile:
                    epilogue_a(s)

        for i in range(NT + 7):
            if i < NT:
                load(i)
            if 0 <= i - 4 < NT:
                convert(i - 4)
            if 0 <= i - 5 < NT:
                transpose_stage(i - 5)
            if 0 <= i - 6 < NT:
                mm1_softmax(i - 6)
            if 0 <= i - 7 < NT:
                mm2_stage(i - 7)
        epilogue_b(0)
        epilogue_b(1)

    nc.finalize()
    return nc


def _get_nc():
    if "nc" not in _cache:
        _cache["nc"] = _build()
    return _cache["nc"]


def kernel(x, Wc, C):
    from concourse.bass_utils import run_bass_kernel_spmd

    nc = _get_nc()
    maps = make_inputs(x, Wc, C)
    res = run_bass_kernel_spmd(nc, maps, list(range(N_CORES)))
    outs = [r["out"].reshape(N_SAMP, D * K) for r in res.results]
    return np.concatenate(outs, axis=0)


def make_inputs(x, Wc, C):
    """Host-side prep: shard x over cores, replicate the small constants."""
    import ml_dtypes
    bf16 = ml_dtypes.bfloat16

    x = np.asarray(x, dtype=np.float32)
    wc16 = np.ascontiguousarray(
        np.asarray(Wc, dtype=np.float32).astype(bf16))
    ct = np.ascontiguousarray(np.asarray(C, dtype=np.float32).T)
    ident16 = np.eye(P, dtype=bf16)
    id32 = np.eye(K, dtype=np.float32)
    ones16 = np.ones((P, 2), dtype=bf16)

    per = x.shape[0] // N_CORES
    maps = []
    for i in range(N_CORES):
        xs = np.ascontiguousarray(
            x[i * per:(i + 1) * per].reshape(N_ROWS, D))
        maps.append({"x": xs, "wc16": wc16, "ct": ct, "ident16": ident16,
                     "id32": id32, "ones16": ones16})
    return maps


# revision 4
# speedup vs baseline: 1.5898x; 1.1019x over previous
"""NetVLAD Trainium2 Bass kernel, SPMD over 8 NeuronCores.

Contract: kernel(x, Wc, C) takes the FULL inputs
  x  [16, 56, 56, 512] f32, Wc [512, 32] f32, C [512, 32] f32
and returns the FULL output [16, 16384] f32 (matches reference()).

Sharding: data-parallel over batch — 2 samples per core; Wc/C replicated.

Host prep per core: x is cast to bf16 and shipped in BOTH layouts —
x16 [6272 pix, 512 d] (streamed by mm2) and x16T [512 d, 6272 pix]
(stationary chunks for mm1). Same total HBM bytes as one f32 copy
(12.8MB/core); removes all on-chip transposes and dtype converts, so
the PE runs only 6 matmul pairs per 128-pixel tile.

Per-core per 128-pixel tile (49 tiles; D=512, K=32):
  - mm1: s[n,k] = sum_j x16T[j-chunk].T @ Wc[j-chunk]  (4 bf16 matmuls)
  - softmax over k: ACT Exp, DVE row-sum + reciprocal + scale -> a bf16
  - mm2: ax[k,d] += a.T @ x16 and a_sum[k] += a.T @ ones (PSUM f32)
  - epilogue: vT = axT + C^T * a_sum, PE-transpose to [d,k], fused
    intra+global L2 normalization (global norm of the intra-normalized
    matrix is exactly sqrt(512), folded analytically); Sqrt deferred to
    the end so the ACT Exp/Sqrt tables each load once.
x16T is fetched in 512-pixel macro tiles (1KB contiguous per
descriptor); emission is software-pipelined (mm1 t / mm2 t-2) so the
in-order PE never waits on the softmax chain.
Measured end-to-end rel err vs the f32 reference ~2e-3.
"""
import sys

if '/opt/trn_rl_repo' not in sys.path:
    sys.path.insert(0, '/opt/trn_rl_repo')

from contextlib import ExitStack

import numpy as np

N_PIX = 3136
N_SAMP = 2
N_ROWS = N_PIX * N_SAMP
P = 128
NT = N_ROWS // P      # 49
D = 512
K = 32
DC = D // P           # 4
BOUND_T = N_PIX // P  # 24
BOUND_R = N_PIX - BOUND_T * P  # 64
N_CORES = 8
MW = 512              # xT macro-tile width (pixels)
NM = (N_ROWS + MW - 1) // MW  # 13 macro tiles (last is 128 wide)

_cache = {}


def _build():
    import concourse.bacc as bacc
    import concourse.mybir as mybir
    import concourse.tile as tile
    from concourse.bass import ts

    F32 = mybir.dt.float32
    BF16 = mybir.dt.bfloat16

    nc = bacc.Bacc("TRN2", target_bir_lowering=False, debug=False)

    x16 = nc.declare_dram_parameter("x16", [N_ROWS, D], BF16, isOutput=False)
    x16t = nc.declare_dram_parameter("x16t", [D, N_ROWS], BF16,
                                     isOutput=False)
    wc = nc.declare_dram_parameter("wc16", [D, K], BF16, isOutput=False)
    ct = nc.declare_dram_parameter("ct", [K, D], F32, isOutput=False)
    id32 = nc.declare_dram_parameter("id32", [K, K], F32, isOutput=False)
    ones2 = nc.declare_dram_parameter("ones16", [P, 2], BF16, isOutput=False)
    out = nc.declare_dram_parameter("out", [N_SAMP, DC, P, K], F32,
                                    isOutput=True)
    x16, x16t, wc, ct, out, id32, ones2 = (
        x16.ap(), x16t.ap(), wc.ap(), ct.ap(), out.ap(), id32.ap(),
        ones2.ap())

    with tile.TileContext(nc) as tc, ExitStack() as ctx:
        consts = ctx.enter_context(tc.tile_pool(name="consts", bufs=1))
        xpool = ctx.enter_context(tc.tile_pool(name="xpool", bufs=7))
        xtpool = ctx.enter_context(tc.tile_pool(name="xtpool", bufs=4))
        small = ctx.enter_context(tc.tile_pool(name="small", bufs=3))
        apool = ctx.enter_context(tc.tile_pool(name="apool", bufs=4))
        epil = ctx.enter_context(tc.tile_pool(name="epil", bufs=2))
        ps_sm = ctx.enter_context(tc.tile_pool(name="ps_sm", bufs=2,
                                               space="PSUM"))
        ps_acc = ctx.enter_context(tc.tile_pool(name="ps_acc", bufs=2,
                                                space="PSUM"))

        wc_sb = consts.tile([P, DC, K], BF16)
        nc.sync.dma_start(out=wc_sb, in_=wc.rearrange("(c p) k -> p c k", p=P))
        ct_sb = consts.tile([K, D], F32)
        nc.sync.dma_start(out=ct_sb, in_=ct)
        id32_sb = consts.tile([K, K], F32)
        nc.sync.dma_start(out=id32_sb, in_=id32)
        ones_sb = consts.tile([P, 2], BF16)
        nc.sync.dma_start(out=ones_sb, in_=ones2)

        acc = [ps_acc.tile([K, D], F32, name=f"acc{s}", tag="acc")
               for s in range(N_SAMP)]
        asum_ps = [ps_acc.tile([K, 2], F32, name=f"asumps{s}", tag="asum_ps")
                   for s in range(N_SAMP)]
        started = [False, False]

        xs, xTs, a16s = {}, {}, {}
        ep_state = {}

        def load_x16(t):
            x_t = xpool.tile([P, D], BF16, name="x16_t")
            nc.sync.dma_start(out=x_t, in_=x16[ts(t, P), :])
            xs[t] = x_t

        def load_xT(m):
            w = min(MW, N_ROWS - m * MW)
            xT = xtpool.tile([P, DC, w], BF16, name="xT_m")
            nc.sync.dma_start(
                out=xT,
                in_=x16t[:, m * MW:m * MW + w].rearrange(
                    "(c p) n -> p c n", p=P))
            xTs[m] = xT

        def mm1_softmax(t):
            m, off = divmod(t * P, MW)
            xT = xTs[m]
            s_ps = ps_sm.tile([P, K], F32, name="s_ps", tag="sps")
            for j in range(DC):
                nc.tensor.matmul(s_ps, xT[:, j, off:off + P],
                                 wc_sb[:, j, :],
                                 start=(j == 0), stop=(j == DC - 1))
            exp_sb = small.tile([P, K], F32, name="exp_sb")
            nc.scalar.activation(exp_sb, s_ps,
                                 mybir.ActivationFunctionType.Exp)
            sumx = small.tile([P, 1], F32, name="sumx")
            nc.vector.reduce_sum(sumx, exp_sb, axis=mybir.AxisListType.X)
            rcp = small.tile([P, 1], F32, name="rcp")
            nc.vector.reciprocal(rcp, sumx)
            a16 = apool.tile([P, K], BF16, name="a16")
            nc.vector.tensor_scalar_mul(a16, exp_sb, rcp)
            a16s[t] = a16

        def epilogue_a(s):
            asum_sb = epil.tile([K, 1], F32, name=f"asum{s}", tag="asum")
            nc.vector.tensor_copy(asum_sb, asum_ps[s][:, 0:1])
            vt_sb = epil.tile([K, D], F32, name=f"vt{s}", tag="vt")
            nc.vector.tensor_scalar_mul(vt_sb, ct_sb, asum_sb)
            nc.vector.tensor_add(vt_sb, vt_sb, acc[s][:, :])
            v_ps = ps_sm.tile([P, DC, K], F32, name=f"vps{s}", tag="sps")
            for j in range(DC):
                nc.tensor.transpose(v_ps[:, j, :], vt_sb[:, ts(j, P)],
                                    id32_sb)
            v_sb = epil.tile([P, DC, K], F32, name=f"v{s}", tag="v")
            nc.vector.tensor_copy(v_sb, v_ps)
            vsq = epil.tile([P, DC, K], F32, name=f"vsq{s}", tag="vsq")
            nc.vector.tensor_mul(vsq, v_sb, v_sb)
            ssq = epil.tile([P, DC], F32, name=f"ssq{s}", tag="ssq")
            nc.vector.reduce_sum(ssq, vsq, axis=mybir.AxisListType.X)
            ep_state[s] = (v_sb, ssq)

        def epilogue_b(s):
            v_sb, ssq = ep_state[s]
            snorm = epil.tile([P, DC], F32, name=f"sn{s}", tag="sn")
            nc.scalar.activation(snorm, ssq,
                                 mybir.ActivationFunctionType.Sqrt,
                                 scale=float(D))
            rmult = epil.tile([P, DC], F32, name=f"rm{s}", tag="rm")
            nc.vector.reciprocal(rmult, snorm)
            for j in range(DC):
                nc.vector.tensor_scalar_mul(v_sb[:, j, :], v_sb[:, j, :],
                                            rmult[:, j:j + 1])
            nc.sync.dma_start(out=out[s].rearrange("c p k -> p c k"),
                              in_=v_sb)

        def mm2_stage(t):
            if t < BOUND_T:
                parts = [(0, 0, P)]
            elif t == BOUND_T:
                parts = [(0, 0, BOUND_R), (1, BOUND_R, P)]
            else:
                parts = [(1, 0, P)]
            a16 = a16s[t]
            for s, r0, r1 in parts:
                first = not started[s]
                started[s] = True
                last_tile = (t == BOUND_T and s == 0) or \
                            (t == NT - 1 and s == 1)
                nc.tensor.matmul(acc[s][:, :], a16[r0:r1, :],
                                 xs[t][r0:r1, :],
                                 start=first, stop=last_tile,
                                 skip_group_check=True)
                nc.tensor.matmul(asum_ps[s][:, :], a16[r0:r1, :],
                                 ones_sb[r0:r1, :],
                                 start=first, stop=last_tile,
                                 skip_group_check=True)
                if last_tile:
                    epilogue_a(s)

        # prologue: prefetch 2 xT macros and 4 x16 tiles
        load_xT(0)
        load_xT(1)
        for t in range(4):
            load_x16(t)

        for i in range(NT + 2):
            if i % 4 == 0 and i // 4 + 2 < NM:
                load_xT(i // 4 + 2)
            if i + 4 < NT:
                load_x16(i + 4)
            if i < NT:
                mm1_softmax(i)
            if 0 <= i - 2 < NT:
                mm2_stage(i - 2)
        epilogue_b(0)
        epilogue_b(1)

    nc.finalize()
    return nc


def _get_nc():
    if "nc" not in _cache:
        _cache["nc"] = _build()
    return _cache["nc"]


def kernel(x, Wc, C):
    from concourse.bass_utils import run_bass_kernel_spmd

    nc = _get_nc()
    maps = make_inputs(x, Wc, C)
    res = run_bass_kernel_spmd(nc, maps, list(range(N_CORES)))
    outs = [r["out"].reshape(N_SAMP, D * K) for r in res.results]
    return np.concatenate(outs, axis=0)


def make_inputs(x, Wc, C):
    """Host-side prep: shard + cast x to bf16 in both layouts."""
    import ml_dtypes
    bf16 = ml_dtypes.bfloat16

    x = np.asarray(x, dtype=np.float32)
    wc16 = np.ascontiguousarray(
        np.asarray(Wc, dtype=np.float32).astype(bf16))
    ct = np.ascontiguousarray(np.asarray(C, dtype=np.float32).T)
    id32 = np.eye(K, dtype=np.float32)
    ones16 = np.ones((P, 2), dtype=bf16)

    per = x.shape[0] // N_CORES
    maps = []
    for i in range(N_CORES):
        xs = x[i * per:(i + 1) * per].reshape(N_ROWS, D).astype(bf16)
        xs = np.ascontiguousarray(xs)
        xst = np.ascontiguousarray(xs.T)
        maps.append({"x16": xs, "x16t": xst, "wc16": wc16, "ct": ct,
                     "id32": id32, "ones16": ones16})
    return maps


# revision 5
# speedup vs baseline: 2.0763x; 1.3060x over previous
"""NetVLAD Trainium2 Bass kernel, SPMD over 8 NeuronCores.

Contract: kernel(x, Wc, C) takes the FULL inputs
  x  [16, 56, 56, 512] f32, Wc [512, 32] f32, C [512, 32] f32
and returns the FULL output [16, 16384] f32 (matches reference()).

Sharding: data-parallel over batch — 2 samples per core; Wc/C replicated.

Host prep per core: x is cast to bf16 and shipped in BOTH layouts —
x16 [6272 pix, 512 d] (streamed by mm2) and x16T [512 d, 6272 pix]
(stationary chunks for mm1). Same total HBM bytes as one f32 copy
(12.8MB/core); removes all on-chip transposes and dtype converts, so
the PE runs only 6 matmul pairs per 128-pixel tile.

Per-core per 128-pixel tile (49 tiles; D=512, K=32):
  - mm1: s[n,k] = sum_j x16T[j-chunk].T @ Wc[j-chunk]  (4 bf16 matmuls)
  - softmax over k, batched over tile PAIRS to halve ACT/DVE
    instruction count: ACT Exp on [128,2,32], DVE row-sum + reciprocal,
    2x scale -> a bf16
  - mm2: ax[k,d] += a.T @ x16 and a_sum[k] += a.T @ ones (PSUM f32)
  - epilogue: vT = axT + C^T * a_sum, PE-transpose to [d,k], fused
    intra+global L2 normalization; 1/sqrt(D*ssq) is computed as
    exp(-0.5*ln(D*ssq)) so the whole kernel uses a single ACT table
    (exp+ln) — no mid-kernel table reloads.
x16T is fetched in 512-pixel macro tiles (1KB contiguous per
descriptor); emission is software-pipelined (mm1 t / mm2 t-3, x16
prefetch 6 tiles, x16T prefetch 3 macros) so the in-order PE never
waits on the softmax chain.
Measured end-to-end rel err vs the f32 reference ~2e-3.
"""
import sys

if '/opt/trn_rl_repo' not in sys.path:
    sys.path.insert(0, '/opt/trn_rl_repo')

from contextlib import ExitStack

import numpy as np

N_PIX = 3136
N_SAMP = 2
N_ROWS = N_PIX * N_SAMP
P = 128
NT = N_ROWS // P      # 49
D = 512
K = 32
DC = D // P           # 4
BOUND_T = N_PIX // P  # 24
BOUND_R = N_PIX - BOUND_T * P  # 64
N_CORES = 8
MW = 512              # xT macro-tile width (pixels)
NM = (N_ROWS + MW - 1) // MW  # 13 macro tiles (last is 128 wide)

_cache = {}


def _build():
    import concourse.bacc as bacc
    import concourse.mybir as mybir
    import concourse.tile as tile
    from concourse.bass import ts

    F32 = mybir.dt.float32
    BF16 = mybir.dt.bfloat16

    nc = bacc.Bacc("TRN2", target_bir_lowering=False, debug=False)

    x16 = nc.declare_dram_parameter("x16", [N_ROWS, D], BF16, isOutput=False)
    x16t = nc.declare_dram_parameter("x16t", [D, N_ROWS], BF16,
                                     isOutput=False)
    wc = nc.declare_dram_parameter("wc16", [D, K], BF16, isOutput=False)
    ct = nc.declare_dram_parameter("ct", [K, D], F32, isOutput=False)
    id32 = nc.declare_dram_parameter("id32", [K, K], F32, isOutput=False)
    ones2 = nc.declare_dram_parameter("ones16", [P, 2], BF16, isOutput=False)
    out = nc.declare_dram_parameter("out", [N_SAMP, DC, P, K], F32,
                                    isOutput=True)
    x16, x16t, wc, ct, out, id32, ones2 = (
        x16.ap(), x16t.ap(), wc.ap(), ct.ap(), out.ap(), id32.ap(),
        ones2.ap())

    with tile.TileContext(nc) as tc, ExitStack() as ctx:
        consts = ctx.enter_context(tc.tile_pool(name="consts", bufs=1))
        xpool = ctx.enter_context(tc.tile_pool(name="xpool", bufs=12))
        xtpool = ctx.enter_context(tc.tile_pool(name="xtpool", bufs=5))
        small = ctx.enter_context(tc.tile_pool(name="small", bufs=3))
        apool = ctx.enter_context(tc.tile_pool(name="apool", bufs=3))
        epil = ctx.enter_context(tc.tile_pool(name="epil", bufs=2))
        ps_sm = ctx.enter_context(tc.tile_pool(name="ps_sm", bufs=2,
                                               space="PSUM"))
        ps_acc = ctx.enter_context(tc.tile_pool(name="ps_acc", bufs=2,
                                                space="PSUM"))

        wc_sb = consts.tile([P, DC, K], BF16)
        nc.sync.dma_start(out=wc_sb, in_=wc.rearrange("(c p) k -> p c k", p=P))
        ct_sb = consts.tile([K, D], F32)
        nc.sync.dma_start(out=ct_sb, in_=ct)
        id32_sb = consts.tile([K, K], F32)
        nc.sync.dma_start(out=id32_sb, in_=id32)
        ones_sb = consts.tile([P, 2], BF16)
        nc.sync.dma_start(out=ones_sb, in_=ones2)

        acc = [ps_acc.tile([K, D], F32, name=f"acc{s}", tag="acc")
               for s in range(N_SAMP)]
        asum_ps = [ps_acc.tile([K, 2], F32, name=f"asumps{s}", tag="asum_ps")
                   for s in range(N_SAMP)]
        started = [False, False]

        xs, xTs, s_pairs, a_pairs = {}, {}, {}, {}
        ep_state = {}

        def load_x16(t):
            x_t = xpool.tile([P, D], BF16, name="x16_t")
            nc.sync.dma_start(out=x_t, in_=x16[ts(t, P), :])
            xs[t] = x_t

        def load_xT(m):
            w = min(MW, N_ROWS - m * MW)
            xT = xtpool.tile([P, DC, w], BF16, name="xT_m")
            nc.sync.dma_start(
                out=xT,
                in_=x16t[:, m * MW:m * MW + w].rearrange(
                    "(c p) n -> p c n", p=P))
            xTs[m] = xT

        def mm1(t):
            m, off = divmod(t * P, MW)
            xT = xTs[m]
            p, q = divmod(t, 2)
            if q == 0:
                s_pairs[p] = ps_sm.tile([P, 2, K], F32, name="s_ps",
                                        tag="sps")
            s_ps = s_pairs[p]
            for j in range(DC):
                nc.tensor.matmul(s_ps[:, q, :], xT[:, j, off:off + P],
                                 wc_sb[:, j, :],
                                 start=(j == 0), stop=(j == DC - 1))

        def softmax_pair(p):
            w = min(2, NT - 2 * p)
            s_ps = s_pairs[p]
            exp_sb = small.tile([P, 2, K], F32, name="exp_sb")
            nc.scalar.activation(exp_sb[:, 0:w, :], s_ps[:, 0:w, :],
                                 mybir.ActivationFunctionType.Exp)
            sumx = small.tile([P, 2], F32, name="sumx")
            nc.vector.reduce_sum(sumx[:, 0:w], exp_sb[:, 0:w, :],
                                 axis=mybir.AxisListType.X)
            rcp = small.tile([P, 2], F32, name="rcp")
            nc.vector.reciprocal(rcp[:, 0:w], sumx[:, 0:w])
            a16 = apool.tile([P, 2, K], BF16, name="a16")
            for q in range(w):
                nc.vector.tensor_scalar_mul(a16[:, q, :], exp_sb[:, q, :],
                                            rcp[:, q:q + 1])
            a_pairs[p] = a16

        def epilogue_a(s):
            asum_sb = epil.tile([K, 1], F32, name=f"asum{s}", tag="asum")
            nc.vector.tensor_copy(asum_sb, asum_ps[s][:, 0:1])
            vt_sb = epil.tile([K, D], F32, name=f"vt{s}", tag="vt")
            nc.vector.tensor_scalar_mul(vt_sb, ct_sb, asum_sb)
            nc.vector.tensor_add(vt_sb, vt_sb, acc[s][:, :])
            v_ps = ps_sm.tile([P, DC, K], F32, name=f"vps{s}", tag="sps")
            for j in range(DC):
                nc.tensor.transpose(v_ps[:, j, :], vt_sb[:, ts(j, P)],
                                    id32_sb)
            v_sb = epil.tile([P, DC, K], F32, name=f"v{s}", tag="v")
            nc.vector.tensor_copy(v_sb, v_ps)
            vsq = epil.tile([P, DC, K], F32, name=f"vsq{s}", tag="vsq")
            nc.vector.tensor_mul(vsq, v_sb, v_sb)
            ssq = epil.tile([P, DC], F32, name=f"ssq{s}", tag="ssq")
            nc.vector.reduce_sum(ssq, vsq, axis=mybir.AxisListType.X)
            ep_state[s] = (v_sb, ssq)

        def epilogue_b(s):
            v_sb, ssq = ep_state[s]
            # rmult = (D*ssq)^-0.5 via exp(-0.5*ln(D*ssq)): stays in the
            # exp+ln ACT table, avoiding a Sqrt table reload.
            lnv = epil.tile([P, DC], F32, name=f"ln{s}", tag="sn")
            nc.scalar.activation(lnv, ssq,
                                 mybir.ActivationFunctionType.Ln,
                                 scale=float(D))
            rmult = epil.tile([P, DC], F32, name=f"rm{s}", tag="rm")
            nc.scalar.activation(rmult, lnv,
                                 mybir.ActivationFunctionType.Exp,
                                 scale=-0.5)
            for j in range(DC):
                nc.vector.tensor_scalar_mul(v_sb[:, j, :], v_sb[:, j, :],
                                            rmult[:, j:j + 1])
            nc.sync.dma_start(out=out[s].rearrange("c p k -> p c k"),
                              in_=v_sb)

        def mm2_stage(t):
            if t < BOUND_T:
                parts = [(0, 0, P)]
            elif t == BOUND_T:
                parts = [(0, 0, BOUND_R), (1, BOUND_R, P)]
            else:
                parts = [(1, 0, P)]
            a16 = a_pairs[t // 2][:, t % 2, :]
            for s, r0, r1 in parts:
                first = not started[s]
                started[s] = True
                last_tile = (t == BOUND_T and s == 0) or \
                            (t == NT - 1 and s == 1)
                nc.tensor.matmul(acc[s][:, :], a16[r0:r1, :],
                                 xs[t][r0:r1, :],
                                 start=first, stop=last_tile,
                                 skip_group_check=True)
                nc.tensor.matmul(asum_ps[s][:, :], a16[r0:r1, :],
                                 ones_sb[r0:r1, :],
                                 start=first, stop=last_tile,
                                 skip_group_check=True)
                if last_tile:
                    epilogue_a(s)

        # prologue: prefetch 3 xT macros and 6 x16 tiles
        for m in range(3):
            load_xT(m)
        for t in range(6):
            load_x16(t)

        for i in range(NT + 3):
            if i % 4 == 0 and i // 4 + 3 < NM:
                load_xT(i // 4 + 3)
            if i + 6 < NT:
                load_x16(i + 6)
            if i < NT:
                mm1(i)
                if i % 2 == 1 or i == NT - 1:
                    softmax_pair(i // 2)
            if 0 <= i - 3 < NT:
                mm2_stage(i - 3)
        epilogue_b(0)
        epilogue_b(1)

    nc.finalize()
    return nc


def _get_nc():
    if "nc" not in _cache:
        _cache["nc"] = _build()
    return _cache["nc"]


def kernel(x, Wc, C):
    from concourse.bass_utils import run_bass_kernel_spmd

    nc = _get_nc()
    maps = make_inputs(x, Wc, C)
    res = run_bass_kernel_spmd(nc, maps, list(range(N_CORES)))
    outs = [r["out"].reshape(N_SAMP, D * K) for r in res.results]
    return np.concatenate(outs, axis=0)


def make_inputs(x, Wc, C):
    """Host-side prep: shard + cast x to bf16 in both layouts."""
    import ml_dtypes
    bf16 = ml_dtypes.bfloat16

    x = np.asarray(x, dtype=np.float32)
    wc16 = np.ascontiguousarray(
        np.asarray(Wc, dtype=np.float32).astype(bf16))
    ct = np.ascontiguousarray(np.asarray(C, dtype=np.float32).T)
    id32 = np.eye(K, dtype=np.float32)
    ones16 = np.ones((P, 2), dtype=bf16)

    per = x.shape[0] // N_CORES
    maps = []
    for i in range(N_CORES):
        xs = x[i * per:(i + 1) * per].reshape(N_ROWS, D).astype(bf16)
        xs = np.ascontiguousarray(xs)
        xst = np.ascontiguousarray(xs.T)
        maps.append({"x16": xs, "x16t": xst, "wc16": wc16, "ct": ct,
                     "id32": id32, "ones16": ones16})
    return maps


# revision 9
# speedup vs baseline: 2.0845x; 1.0040x over previous
"""NetVLAD Trainium2 Bass kernel, SPMD over 8 NeuronCores.

Contract: kernel(x, Wc, C) takes the FULL inputs
  x  [16, 56, 56, 512] f32, Wc [512, 32] f32, C [512, 32] f32
and returns the FULL output [16, 16384] f32 (matches reference()).

Sharding: data-parallel over batch — 2 samples per core; Wc/C replicated.

Host prep per core: x is cast to bf16 and shipped in BOTH layouts —
x16 [6272 pix, 512 d] (streamed by mm2) and x16T [512 d, 6272 pix]
(stationary chunks for mm1). Same total HBM bytes as one f32 copy
(12.8MB/core); removes all on-chip transposes and dtype converts, so
the PE runs only 6 matmul pairs per 128-pixel tile.

Per-core per 128-pixel tile (49 tiles; D=512, K=32):
  - mm1: s[n,k] = sum_j x16T[j-chunk].T @ Wc[j-chunk]  (4 bf16 matmuls)
  - softmax over k, batched over tile PAIRS to halve ACT/DVE
    instruction count: ACT Exp on [128,2,32], DVE row-sum + reciprocal,
    2x scale -> a bf16
  - mm2: ax[k,d] += a.T @ x16 and a_sum[k] += a.T @ ones (PSUM f32)
  - epilogue: vT = axT + C^T * a_sum, PE-transpose to [d,k], fused
    intra+global L2 normalization; 1/sqrt(D*ssq) is computed as
    exp(-0.5*ln(D*ssq)) so the whole kernel uses a single ACT table
    (exp+ln) — no mid-kernel table reloads.
x16T is fetched in 512-pixel macro tiles (1KB contiguous per
descriptor); emission is software-pipelined (mm1 t / mm2 t-3, x16
prefetch 6 tiles, x16T prefetch 3 macros) so the in-order PE never
waits on the softmax chain.
Measured end-to-end rel err vs the f32 reference ~2e-3.
"""
import sys

if '/opt/trn_rl_repo' not in sys.path:
    sys.path.insert(0, '/opt/trn_rl_repo')

from contextlib import ExitStack

import numpy as np

N_PIX = 3136
N_SAMP = 2
N_ROWS = N_PIX * N_SAMP
P = 128
NT = N_ROWS // P      # 49
D = 512
K = 32
DC = D // P           # 4
BOUND_T = N_PIX // P  # 24
BOUND_R = N_PIX - BOUND_T * P  # 64
N_CORES = 8
MW = 512              # xT macro-tile width (pixels)
NM = (N_ROWS + MW - 1) // MW  # 13 macro tiles (last is 128 wide)

_cache = {}


def _build():
    import concourse.bacc as bacc
    import concourse.mybir as mybir
    import concourse.tile as tile
    from concourse.bass import ts

    F32 = mybir.dt.float32
    BF16 = mybir.dt.bfloat16

    nc = bacc.Bacc("TRN2", target_bir_lowering=False, debug=False)

    x16 = nc.declare_dram_parameter("x16", [N_ROWS, D], BF16, isOutput=False)
    x16t = nc.declare_dram_parameter("x16t", [D, N_ROWS], BF16,
                                     isOutput=False)
    wc = nc.declare_dram_parameter("wc16", [D, K], BF16, isOutput=False)
    ct = nc.declare_dram_parameter("ct", [K, D], F32, isOutput=False)
    id32 = nc.declare_dram_parameter("id32", [K, K], F32, isOutput=False)
    ones2 = nc.declare_dram_parameter("ones16", [P, 2], BF16, isOutput=False)
    out = nc.declare_dram_parameter("out", [N_SAMP, DC, P, K], F32,
                                    isOutput=True)
    x16, x16t, wc, ct, out, id32, ones2 = (
        x16.ap(), x16t.ap(), wc.ap(), ct.ap(), out.ap(), id32.ap(),
        ones2.ap())

    with tile.TileContext(nc) as tc, ExitStack() as ctx:
        consts = ctx.enter_context(tc.tile_pool(name="consts", bufs=1))
        xpool = ctx.enter_context(tc.tile_pool(name="xpool", bufs=5))
        xtpool = ctx.enter_context(tc.tile_pool(name="xtpool", bufs=5))
        small = ctx.enter_context(tc.tile_pool(name="small", bufs=3))
        apool = ctx.enter_context(tc.tile_pool(name="apool", bufs=3))
        epil = ctx.enter_context(tc.tile_pool(name="epil", bufs=2))
        ps_sm = ctx.enter_context(tc.tile_pool(name="ps_sm", bufs=2,
                                               space="PSUM"))
        ps_acc = ctx.enter_context(tc.tile_pool(name="ps_acc", bufs=2,
                                                space="PSUM"))

        wc_sb = consts.tile([P, DC, K], BF16)
        nc.sync.dma_start(out=wc_sb, in_=wc.rearrange("(c p) k -> p c k", p=P))
        ct_sb = consts.tile([K, D], F32)
        nc.sync.dma_start(out=ct_sb, in_=ct)
        id32_sb = consts.tile([K, K], F32)
        nc.sync.dma_start(out=id32_sb, in_=id32)
        ones_sb = consts.tile([P, 2], BF16)
        nc.sync.dma_start(out=ones_sb, in_=ones2)

        acc = [ps_acc.tile([K, D], F32, name=f"acc{s}", tag="acc")
               for s in range(N_SAMP)]
        asum_ps = [ps_acc.tile([K, 2], F32, name=f"asumps{s}", tag="asum_ps")
                   for s in range(N_SAMP)]
        started = [False, False]

        xms, xTs, s_pairs, a_pairs = {}, {}, {}, {}
        ep_state = {}

        def load_x16(m):
            w = min(MW, N_ROWS - m * MW) // P
            x_m = xpool.tile([P, w, D], BF16, name="x16_m")
            nc.sync.dma_start(
                out=x_m,
                in_=x16[m * MW:m * MW + w * P, :].rearrange(
                    "(q p) d -> p q d", p=P))
            xms[m] = x_m

        def load_xT(m):
            w = min(MW, N_ROWS - m * MW)
            xT = xtpool.tile([P, DC, w], BF16, name="xT_m")
            nc.sync.dma_start(
                out=xT,
                in_=x16t[:, m * MW:m * MW + w].rearrange(
                    "(c p) n -> p c n", p=P))
            xTs[m] = xT

        def mm1(t):
            m, off = divmod(t * P, MW)
            xT = xTs[m]
            p, q = divmod(t, 2)
            if q == 0:
                s_pairs[p] = ps_sm.tile([P, 2, K], F32, name="s_ps",
                                        tag="sps")
            s_ps = s_pairs[p]
            for j in range(DC):
                nc.tensor.matmul(s_ps[:, q, :], xT[:, j, off:off + P],
                                 wc_sb[:, j, :],
                                 start=(j == 0), stop=(j == DC - 1))

        def softmax_pair(p):
            w = min(2, NT - 2 * p)
            s_ps = s_pairs[p]
            exp_sb = small.tile([P, 2, K], F32, name="exp_sb")
            nc.scalar.activation(exp_sb[:, 0:w, :], s_ps[:, 0:w, :],
                                 mybir.ActivationFunctionType.Exp)
            sumx = small.tile([P, 2], F32, name="sumx")
            nc.vector.reduce_sum(sumx[:, 0:w], exp_sb[:, 0:w, :],
                                 axis=mybir.AxisListType.X)
            rcp = small.tile([P, 2], F32, name="rcp")
            nc.vector.reciprocal(rcp[:, 0:w], sumx[:, 0:w])
            a16 = apool.tile([P, 2, K], BF16, name="a16")
            for q in range(w):
                nc.vector.tensor_scalar_mul(a16[:, q, :], exp_sb[:, q, :],
                                            rcp[:, q:q + 1])
            a_pairs[p] = a16

        def epilogue_a(s):
            asum_sb = epil.tile([K, 1], F32, name=f"asum{s}", tag="asum")
            nc.vector.tensor_copy(asum_sb, asum_ps[s][:, 0:1])
            vt_sb = epil.tile([K, D], F32, name=f"vt{s}", tag="vt")
            nc.vector.tensor_scalar_mul(vt_sb, ct_sb, asum_sb)
            nc.vector.tensor_add(vt_sb, vt_sb, acc[s][:, :])
            v_ps = ps_sm.tile([P, DC, K], F32, name=f"vps{s}", tag="sps")
            for j in range(DC):
                nc.tensor.transpose(v_ps[:, j, :], vt_sb[:, ts(j, P)],
                                    id32_sb)
            v_sb = epil.tile([P, DC, K], F32, name=f"v{s}", tag="v")
            nc.vector.tensor_copy(v_sb, v_ps)
            vsq = epil.tile([P, DC, K], F32, name=f"vsq{s}", tag="vsq")
            nc.vector.tensor_mul(vsq, v_sb, v_sb)
            ssq = epil.tile([P, DC], F32, name=f"ssq{s}", tag="ssq")
            nc.vector.reduce_sum(ssq, vsq, axis=mybir.AxisListType.X)
            ep_state[s] = (v_sb, ssq)

        def epilogue_b(s):
            v_sb, ssq = ep_state[s]
            # rmult = (D*ssq)^-0.5 via exp(-0.5*ln(D*ssq)): stays in the
            # exp+ln ACT table, avoiding a Sqrt table reload.
            lnv = epil.tile([P, DC], F32, name=f"ln{s}", tag="sn")
            nc.scalar.activation(lnv, ssq,
                                 mybir.ActivationFunctionType.Ln,
                                 scale=float(D))
            rmult = epil.tile([P, DC], F32, name=f"rm{s}", tag="rm")
            nc.scalar.activation(rmult, lnv,
                                 mybir.ActivationFunctionType.Exp,
                                 scale=-0.5)
            for j in range(DC):
                nc.vector.tensor_scalar_mul(v_sb[:, j, :], v_sb[:, j, :],
                                            rmult[:, j:j + 1])
            nc.sync.dma_start(out=out[s].rearrange("c p k -> p c k"),
                              in_=v_sb)

        def mm2_stage(t):
            if t < BOUND_T:
                parts = [(0, 0, P)]
            elif t == BOUND_T:
                parts = [(0, 0, BOUND_R), (1, BOUND_R, P)]
            else:
                parts = [(1, 0, P)]
            a16 = a_pairs[t // 2][:, t % 2, :]
            x_t = xms[t // 4][:, t % 4, :]
            for s, r0, r1 in parts:
                first = not started[s]
                started[s] = True
                last_tile = (t == BOUND_T and s == 0) or \
                            (t == NT - 1 and s == 1)
                nc.tensor.matmul(acc[s][:, :], a16[r0:r1, :],
                                 x_t[r0:r1, :],
                                 start=first, stop=last_tile,
                                 skip_group_check=True)
                nc.tensor.matmul(asum_ps[s][:, :], a16[r0:r1, :],
                                 ones_sb[r0:r1, :],
                                 start=first, stop=last_tile,
                                 skip_group_check=True)
                if last_tile:
                    epilogue_a(s)

        # prologue: prefetch 3 xT macros and 2 x16 macros
        for m in range(3):
            load_xT(m)
        for m in range(2):
            load_x16(m)

        for i in range(NT + 3):
            if i % 4 == 0:
                if i // 4 + 3 < NM:
                    load_xT(i // 4 + 3)
                if i // 4 + 2 < NM:
                    load_x16(i // 4 + 2)
            if i < NT:
                mm1(i)
                if i % 2 == 1 or i == NT - 1:
                    softmax_pair(i // 2)
            if 0 <= i - 3 < NT:
                mm2_stage(i - 3)
        epilogue_b(0)
        epilogue_b(1)

    nc.finalize()
    return nc


def _get_nc():
    if "nc" not in _cache:
        _cache["nc"] = _build()
    return _cache["nc"]


def kernel(x, Wc, C):
    from concourse.bass_utils import run_bass_kernel_spmd

    nc = _get_nc()
    maps = make_inputs(x, Wc, C)
    res = run_bass_kernel_spmd(nc, maps, list(range(N_CORES)))
    outs = [r["out"].reshape(N_SAMP, D * K) for r in res.results]
    return np.concatenate(outs, axis=0)


def make_inputs(x, Wc, C):
    """Host-side prep: shard + cast x to bf16 in both layouts."""
    import ml_dtypes
    bf16 = ml_dtypes.bfloat16

    x = np.asarray(x, dtype=np.float32)
    wc16 = np.ascontiguousarray(
        np.asarray(Wc, dtype=np.float32).astype(bf16))
    ct = np.ascontiguousarray(np.asarray(C, dtype=np.float32).T)
    id32 = np.eye(K, dtype=np.float32)
    ones16 = np.ones((P, 2), dtype=bf16)

    per = x.shape[0] // N_CORES
    maps = []
    for i in range(N_CORES):
        xs = x[i * per:(i + 1) * per].reshape(N_ROWS, D).astype(bf16)
        xs = np.ascontiguousarray(xs)
        xst = np.ascontiguousarray(xs.T)
        maps.append({"x16": xs, "x16t": xst, "wc16": wc16, "ct": ct,
                     "id32": id32, "ones16": ones16})
    return maps
